# revision 7
# baseline (speedup 1.0000x reference)
"""Multi-headed self-attention (B=8, S=1024, D=768, H=12) on 8 TRN2 cores.

Sharding: data-parallel over batch -- core i computes batch element i.
Per-core kernel (all operands pre-transposed on host):
    Qt = (Wq @ x.T + bq)      [D, S]   (o on partitions)
    Kt = (Wk @ x.T + bk)      [D, S]
    V  = (x @ Wv.T + bv)      [S, D]   augmented with a ones column per head
    St_h = Kt_h^T-slices @ Qt_h   -> scores transposed [k, q]
    Et = exp(St/8 + maskbias[k])  (ACT, mask bias per-partition)
    PVt'_h = V'_h.T @ Et_h        [65, q]; row 64 = sum_k Et = Z[q]
    out_h.T = PVt'_h[0:64] / Z    -> outT rows h*64..h*64+63
Host transposes outT back.
"""

import numpy as np

import concourse.bacc as bacc
import concourse.tile as tile
from concourse import mybir
from concourse.bass_utils import run_bass_kernel_spmd

B, S, D, H = 8, 1024, 768, 12
HD = D // H  # 64
N_CORES = 8
SC = S // 128  # 8 key/seq chunks
OC = D // 128  # 6 output chunks (2 heads each)
DC = D // 128  # 6 contraction chunks
NT = 512  # matmul moving-dim tile (fp32 max)
QT = S // NT  # 2
F32 = mybir.dt.float32
F32R = mybir.dt.float32r

HW = HD + 1  # per-head V width incl. ones column


def build():
    nc = bacc.Bacc("TRN2", target_bir_lowering=False, debug=False, num_devices=N_CORES)
    xT = nc.dram_tensor("xT", [D, S], F32R, kind="ExternalInput").ap()
    wqT = nc.dram_tensor("wqT", [D, D], F32R, kind="ExternalInput").ap()
    wkT = nc.dram_tensor("wkT", [D, D], F32R, kind="ExternalInput").ap()
    wvT = nc.dram_tensor("wvT", [D, D], F32R, kind="ExternalInput").ap()
    bq = nc.dram_tensor("bq", [D], F32, kind="ExternalInput").ap()
    bk = nc.dram_tensor("bk", [D], F32, kind="ExternalInput").ap()
    bvb = nc.dram_tensor("bvb", [128, D], F32, kind="ExternalInput").ap()
    mb = nc.dram_tensor("mb", [S], F32, kind="ExternalInput").ap()
    outT = nc.dram_tensor("outT", [D, S], F32, kind="ExternalOutput").ap()

    with tile.TileContext(nc) as tc:
        with (
            tc.tile_pool(name="const", bufs=1) as const,
            tc.tile_pool(name="qk", bufs=2) as qk_pool,
            tc.tile_pool(name="et", bufs=6) as et_pool,
            tc.tile_pool(name="epi", bufs=2) as epi_pool,
            tc.tile_pool(name="mm", bufs=2, space="PSUM") as mm_ps,
            tc.tile_pool(name="pv", bufs=2, space="PSUM") as pv_ps,
            tc.tile_pool(name="dram", bufs=2, space="DRAM") as dram_pool,
        ):
            # ---------- constant / weight loads ----------
            xt = [const.tile([128, S], F32R, tag=f"xt{c}", name=f"xt{c}") for c in range(DC)]
            for c in range(DC):
                nc.sync.dma_start(xt[c][:], xT[c * 128:(c + 1) * 128, :])
            wq = [const.tile([128, D], F32R, tag=f"wq{c}", name=f"wq{c}") for c in range(DC)]
            wk = [const.tile([128, D], F32R, tag=f"wk{c}", name=f"wk{c}") for c in range(DC)]
            wv = [const.tile([128, D], F32R, tag=f"wv{c}", name=f"wv{c}") for c in range(DC)]
            for w_t, w_ap in ((wq, wqT), (wk, wkT), (wv, wvT)):
                for c in range(DC):
                    nc.sync.dma_start(w_t[c][:], w_ap[c * 128:(c + 1) * 128, :])
            bq_t = const.tile([128, OC], F32, tag="bq")
            nc.sync.dma_start(bq_t[:], bq.rearrange("(c p) -> p c", p=128))
            bk_t = const.tile([128, OC], F32, tag="bk")
            nc.sync.dma_start(bk_t[:], bk.rearrange("(c p) -> p c", p=128))
            bvb_t = const.tile([128, D], F32, tag="bvb")
            nc.sync.dma_start(bvb_t[:], bvb[:])
            mb_t = const.tile([128, SC], F32, tag="mb")
            nc.sync.dma_start(mb_t[:], mb.rearrange("(c p) -> p c", p=128))

            # ---------- V projection -> vaug [sc][128, H*65] ----------
            vaug = [const.tile([128, H * HW], F32R, tag=f"va{sc}", name=f"va{sc}") for sc in range(SC)]
            for sc in range(SC):
                ones_cols = vaug[sc][:].rearrange("p (h w) -> p h w", h=H)[:, :, HD:HW]
                nc.vector.memset(ones_cols.bitcast(F32), 1.0)
            for sc in range(SC):
                vp = mm_ps.tile([128, D], F32, tag="mm")
                for n0 in range(0, D, NT):
                    n1 = min(n0 + NT, D)
                    for c in range(DC):
                        nc.tensor.matmul(
                            vp[:, n0:n1],
                            xt[c][:, sc * 128:(sc + 1) * 128],
                            wv[c][:, n0:n1],
                            start=(c == 0),
                            stop=(c == DC - 1),
                        )
                nc.vector.tensor_add(
                    vaug[sc][:].rearrange("p (h w) -> p h w", h=H)[:, :, 0:HD],
                    vp[:].rearrange("p (h w) -> p h w", h=H),
                    bvb_t[:].rearrange("p (h w) -> p h w", h=H),
                )

            # ---------- per-o-chunk: Q/K projection + attention ----------
            for oc in range(OC):
                qkt = {}
                for name, w_t, b_t in (("q", wq, bq_t), ("k", wk, bk_t)):
                    p = mm_ps.tile([128, S], F32, tag="mm")
                    for qt in range(QT):
                        for c in range(DC):
                            nc.tensor.matmul(
                                p[:, qt * NT:(qt + 1) * NT],
                                w_t[c][:, oc * 128:(oc + 1) * 128],
                                xt[c][:, qt * NT:(qt + 1) * NT],
                                start=(c == 0),
                                stop=(c == DC - 1),
                            )
                    dst = qk_pool.tile([128, S], F32R, tag=name)
                    nc.vector.tensor_scalar_add(dst[:], p[:], b_t[:, oc:oc + 1])
                    qkt[name] = dst

                pvt = {}
                for hh in range(2):
                    pvt[hh] = pv_ps.tile([HW, S], F32, tag="pv", name=f"pvt{hh}")

                for kc in range(SC):
                    st = {}
                    et = {}
                    for hh in range(2):
                        p0 = hh * 64
                        stt = mm_ps.tile([128, S], F32, tag="mm")
                        for qt in range(QT):
                            nc.tensor.matmul(
                                stt[:, qt * NT:(qt + 1) * NT],
                                qkt["k"][p0:p0 + 64, kc * 128:(kc + 1) * 128],
                                qkt["q"][p0:p0 + 64, qt * NT:(qt + 1) * NT],
                                tile_position=(p0, 0),
                            )
                        st[hh] = stt
                    for hh in range(2):
                        ett = et_pool.tile([128, S], F32R, tag="et")
                        nc.scalar.activation(
                            ett[:],
                            st[hh][:],
                            mybir.ActivationFunctionType.Exp,
                            bias=mb_t[:, kc:kc + 1],
                            scale=1.0 / np.sqrt(HD),
                        )
                        et[hh] = ett
                    for hh in range(2):
                        gh = 2 * oc + hh
                        for qt in range(QT):
                            nc.tensor.matmul(
                                pvt[hh][:, qt * NT:(qt + 1) * NT],
                                vaug[kc][:, gh * HW:(gh + 1) * HW],
                                et[hh][:, qt * NT:(qt + 1) * NT],
                                start=(kc == 0),
                                stop=(kc == SC - 1),
                            )

                # ---------- epilogue: normalize by Z and store ----------
                for hh in range(2):
                    gh = 2 * oc + hh
                    rz = epi_pool.tile([1, S], F32, tag="rz")
                    nc.vector.reciprocal(rz[:], pvt[hh][HD:HW, :])
                    rzd = dram_pool.tile([1, S], F32, tag="rzd")
                    nc.sync.dma_start(rzd[:], rz[:])
                    zb = epi_pool.tile([HD, S], F32, tag="zb")
                    nc.sync.dma_start(zb[:], rzd[:].partition_broadcast(HD))
                    oh = epi_pool.tile([HD, S], F32, tag="oh")
                    nc.vector.tensor_mul(oh[:], pvt[hh][0:HD, :], zb[:])
                    nc.sync.dma_start(outT[gh * HD:(gh + 1) * HD, :], oh[:])

    nc.compile()
    return nc


_NC = None


def _get_nc():
    global _NC
    if _NC is None:
        _NC = build()
    return _NC


def _in_maps(x, mask, Wq, bq, Wk, bk, Wv, bv):
    x = np.asarray(x, dtype=np.float32)
    mask = np.asarray(mask)
    wqT = np.ascontiguousarray(np.asarray(Wq, dtype=np.float32).T)
    wkT = np.ascontiguousarray(np.asarray(Wk, dtype=np.float32).T)
    wvT = np.ascontiguousarray(np.asarray(Wv, dtype=np.float32).T)
    bq = np.asarray(bq, dtype=np.float32)
    bk = np.asarray(bk, dtype=np.float32)
    bvb = np.ascontiguousarray(
        np.broadcast_to(np.asarray(bv, dtype=np.float32), (128, D))
    )
    maps = []
    for c in range(N_CORES):
        maps.append(
            {
                "xT": np.ascontiguousarray(x[c].T),
                "wqT": wqT,
                "wkT": wkT,
                "wvT": wvT,
                "bq": bq,
                "bk": bk,
                "bvb": bvb,
                "mb": (-10000.0 * (1.0 - mask[c].astype(np.float32))).astype(
                    np.float32
                ),
            }
        )
    return maps


def run(inputs, trace=False, **kw):
    nc = _get_nc()
    res = run_bass_kernel_spmd(
        nc, _in_maps(**inputs), list(range(N_CORES)), trace=trace, **kw
    )
    out = np.stack(
        [np.ascontiguousarray(res.results[c]["outT"].T) for c in range(N_CORES)]
    ).astype(np.float32)
    return out, res


def kernel(**inputs):
    out, _ = run(inputs)
    return out


# revision 9
# speedup vs baseline: 1.1312x; 1.1312x over previous
"""Multi-headed self-attention (B=8, S=1024, D=768, H=12) on 8 TRN2 cores.

Sharding: data-parallel over batch -- core i computes batch element i.
Per-core kernel (all operands pre-transposed on host):
    Qt = (Wq @ x.T + bq)      [D, S]   (o on partitions)
    Kt = (Wk @ x.T + bk)      [D, S]
    V  = (x @ Wv.T + bv)      [S, D]   augmented with a ones column per head
    St_h = Kt_h^T-slices @ Qt_h   -> scores transposed [k, q]
    Et = exp(St/8 + maskbias[k])  (ACT, mask bias per-partition)
    PVt'_h = V'_h.T @ Et_h        [65, q]; row 64 = sum_k Et = Z[q]
    out_h.T = PVt'_h[0:64] / Z    -> outT rows h*64..h*64+63
Host transposes outT back.
"""

import numpy as np

import concourse.bacc as bacc
import concourse.tile as tile
from concourse import mybir
from concourse.bass_utils import run_bass_kernel_spmd

B, S, D, H = 8, 1024, 768, 12
HD = D // H  # 64
N_CORES = 8
SC = S // 128  # 8 key/seq chunks
OC = D // 128  # 6 output chunks (2 heads each)
DC = D // 128  # 6 contraction chunks
NT = 512  # matmul moving-dim tile (fp32 max)
QT = S // NT  # 2
F32 = mybir.dt.float32
F32R = mybir.dt.float32r

HW = HD + 1  # per-head V width incl. ones column


def build():
    nc = bacc.Bacc("TRN2", target_bir_lowering=False, debug=False, num_devices=N_CORES)
    xT = nc.dram_tensor("xT", [D, S], F32R, kind="ExternalInput").ap()
    wqT = nc.dram_tensor("wqT", [D, D], F32R, kind="ExternalInput").ap()
    wkT = nc.dram_tensor("wkT", [D, D], F32R, kind="ExternalInput").ap()
    wvT = nc.dram_tensor("wvT", [D, D], F32R, kind="ExternalInput").ap()
    bq = nc.dram_tensor("bq", [D], F32, kind="ExternalInput").ap()
    bk = nc.dram_tensor("bk", [D], F32, kind="ExternalInput").ap()
    bvb = nc.dram_tensor("bvb", [128, D], F32, kind="ExternalInput").ap()
    mb = nc.dram_tensor("mb", [S], F32, kind="ExternalInput").ap()
    outT = nc.dram_tensor("outT", [D, S], F32, kind="ExternalOutput").ap()

    with tile.TileContext(nc) as tc:
        with (
            tc.tile_pool(name="const", bufs=1) as const,
            tc.tile_pool(name="qk", bufs=2) as qk_pool,
            tc.tile_pool(name="et", bufs=6) as et_pool,
            tc.tile_pool(name="epi", bufs=2) as epi_pool,
            tc.tile_pool(name="mm", bufs=2, space="PSUM") as mm_ps,
            tc.tile_pool(name="pv", bufs=2, space="PSUM") as pv_ps,
            tc.tile_pool(name="dram", bufs=2, space="DRAM") as dram_pool,
        ):
            # ---------- constant / weight loads ----------
            xt = [const.tile([128, S], F32R, tag=f"xt{c}", name=f"xt{c}") for c in range(DC)]
            wq = [const.tile([128, D], F32R, tag=f"wq{c}", name=f"wq{c}") for c in range(DC)]
            wk = [const.tile([128, D], F32R, tag=f"wk{c}", name=f"wk{c}") for c in range(DC)]
            wv = [const.tile([128, D], F32R, tag=f"wv{c}", name=f"wv{c}") for c in range(DC)]
            # interleave so every d-chunk (x, wv, wq, wk) lands early and evenly
            for c in range(DC):
                nc.sync.dma_start(xt[c][:], xT[c * 128:(c + 1) * 128, :])
                nc.sync.dma_start(wv[c][:], wvT[c * 128:(c + 1) * 128, :])
                nc.sync.dma_start(wq[c][:], wqT[c * 128:(c + 1) * 128, :])
                nc.sync.dma_start(wk[c][:], wkT[c * 128:(c + 1) * 128, :])
            bq_t = const.tile([128, OC], F32, tag="bq")
            nc.sync.dma_start(bq_t[:], bq.rearrange("(c p) -> p c", p=128))
            bk_t = const.tile([128, OC], F32, tag="bk")
            nc.sync.dma_start(bk_t[:], bk.rearrange("(c p) -> p c", p=128))
            bvb_t = const.tile([128, D], F32, tag="bvb")
            nc.sync.dma_start(bvb_t[:], bvb[:])
            mb_t = const.tile([128, SC], F32, tag="mb")
            nc.sync.dma_start(mb_t[:], mb.rearrange("(c p) -> p c", p=128))

            # ---------- V projection -> vaug [sc][128, H*65] ----------
            vaug = [const.tile([128, H * HW], F32R, tag=f"va{sc}", name=f"va{sc}") for sc in range(SC)]
            for sc in range(SC):
                ones_cols = vaug[sc][:].rearrange("p (h w) -> p h w", h=H)[:, :, HD:HW]
                nc.vector.memset(ones_cols.bitcast(F32), 1.0)
            for sc in range(SC):
                vp = mm_ps.tile([128, D], F32, tag="mm")
                for n0 in range(0, D, NT):
                    n1 = min(n0 + NT, D)
                    for c in range(DC):
                        nc.tensor.matmul(
                            vp[:, n0:n1],
                            xt[c][:, sc * 128:(sc + 1) * 128],
                            wv[c][:, n0:n1],
                            start=(c == 0),
                            stop=(c == DC - 1),
                        )
                nc.vector.tensor_add(
                    vaug[sc][:].rearrange("p (h w) -> p h w", h=H)[:, :, 0:HD],
                    vp[:].rearrange("p (h w) -> p h w", h=H),
                    bvb_t[:].rearrange("p (h w) -> p h w", h=H),
                )

            # ---------- per-o-chunk: Q/K projection + attention ----------
            for oc in range(OC):
                qkt = {}
                for name, w_t, b_t in (("q", wq, bq_t), ("k", wk, bk_t)):
                    p = mm_ps.tile([128, S], F32, tag="mm")
                    for qt in range(QT):
                        for c in range(DC):
                            nc.tensor.matmul(
                                p[:, qt * NT:(qt + 1) * NT],
                                w_t[c][:, oc * 128:(oc + 1) * 128],
                                xt[c][:, qt * NT:(qt + 1) * NT],
                                start=(c == 0),
                                stop=(c == DC - 1),
                            )
                    dst = qk_pool.tile([128, S], F32R, tag=name)
                    nc.vector.tensor_scalar_add(dst[:], p[:], b_t[:, oc:oc + 1])
                    qkt[name] = dst

                pvt = {}
                for hh in range(2):
                    pvt[hh] = pv_ps.tile([HW, S], F32, tag="pv", name=f"pvt{hh}")

                for kc in range(SC):
                    st = {}
                    et = {}
                    for hh in range(2):
                        p0 = hh * 64
                        stt = mm_ps.tile([128, S], F32, tag="mm")
                        for qt in range(QT):
                            nc.tensor.matmul(
                                stt[:, qt * NT:(qt + 1) * NT],
                                qkt["k"][p0:p0 + 64, kc * 128:(kc + 1) * 128],
                                qkt["q"][p0:p0 + 64, qt * NT:(qt + 1) * NT],
                                tile_position=(p0, 0),
                            )
                        st[hh] = stt
                    for hh in range(2):
                        ett = et_pool.tile([128, S], F32R, tag="et")
                        nc.scalar.activation(
                            ett[:],
                            st[hh][:],
                            mybir.ActivationFunctionType.Exp,
                            bias=mb_t[:, kc:kc + 1],
                            scale=1.0 / np.sqrt(HD),
                        )
                        et[hh] = ett
                    for hh in range(2):
                        gh = 2 * oc + hh
                        for qt in range(QT):
                            nc.tensor.matmul(
                                pvt[hh][:, qt * NT:(qt + 1) * NT],
                                vaug[kc][:, gh * HW:(gh + 1) * HW],
                                et[hh][:, qt * NT:(qt + 1) * NT],
                                start=(kc == 0),
                                stop=(kc == SC - 1),
                            )

                # ---------- epilogue: normalize by Z and store ----------
                # 1/Z must run on a [128, n] layout (DVE reciprocal costs
                # ~6 cyc per element per lane) -> transpose Z via DRAM bounce.
                for hh in range(2):
                    gh = 2 * oc + hh
                    pvs = epi_pool.tile([HW, S], F32, tag="pvs")
                    nc.vector.tensor_copy(pvs[:], pvt[hh][:])
                    zd = dram_pool.tile([S], F32, tag="zd")
                    nc.sync.dma_start(zd[:], pvs[HD:HW, :])
                    zp = epi_pool.tile([128, SC], F32, tag="zp")
                    nc.sync.dma_start(zp[:], zd.rearrange("(c p) -> p c", p=128))
                    nc.vector.reciprocal(zp[:], zp[:])
                    rzd = dram_pool.tile([S], F32, tag="rzd")
                    nc.sync.dma_start(
                        rzd.rearrange("(c p) -> p c", p=128), zp[:]
                    )
                    zb = epi_pool.tile([HD, S], F32, tag="zb")
                    nc.sync.dma_start(zb[:], rzd[:].partition_broadcast(HD))
                    oh = epi_pool.tile([HD, S], F32, tag="oh")
                    nc.vector.tensor_mul(oh[:], pvs[0:HD, :], zb[:])
                    nc.sync.dma_start(outT[gh * HD:(gh + 1) * HD, :], oh[:])

    nc.compile()
    return nc


_NC = None


def _get_nc():
    global _NC
    if _NC is None:
        _NC = build()
    return _NC


def _in_maps(x, mask, Wq, bq, Wk, bk, Wv, bv):
    x = np.asarray(x, dtype=np.float32)
    mask = np.asarray(mask)
    wqT = np.ascontiguousarray(np.asarray(Wq, dtype=np.float32).T)
    wkT = np.ascontiguousarray(np.asarray(Wk, dtype=np.float32).T)
    wvT = np.ascontiguousarray(np.asarray(Wv, dtype=np.float32).T)
    bq = np.asarray(bq, dtype=np.float32)
    bk = np.asarray(bk, dtype=np.float32)
    bvb = np.ascontiguousarray(
        np.broadcast_to(np.asarray(bv, dtype=np.float32), (128, D))
    )
    maps = []
    for c in range(N_CORES):
        maps.append(
            {
                "xT": np.ascontiguousarray(x[c].T),
                "wqT": wqT,
                "wkT": wkT,
                "wvT": wvT,
                "bq": bq,
                "bk": bk,
                "bvb": bvb,
                "mb": (-10000.0 * (1.0 - mask[c].astype(np.float32))).astype(
                    np.float32
                ),
            }
        )
    return maps


def run(inputs, trace=False, **kw):
    nc = _get_nc()
    res = run_bass_kernel_spmd(
        nc, _in_maps(**inputs), list(range(N_CORES)), trace=trace, **kw
    )
    out = np.stack(
        [np.ascontiguousarray(res.results[c]["outT"].T) for c in range(N_CORES)]
    ).astype(np.float32)
    return out, res


def kernel(**inputs):
    out, _ = run(inputs)
    return out
